# revision 9
# baseline (speedup 1.0000x reference)
"""LSTM autoencoder (2-layer enc + autoregressive 2-layer dec + fc) on 8 trn2 cores.

v2: latency-oriented redesign. The recurrence is serial over T; wall time is
dominated by the per-step dependency chain (each cross-engine hop costs
~0.4-1.5us on HW), so the kernel minimizes serialized instructions per step:

  MM(PE) -> tanh gates split in two ACT ops -> u,v (DVE) -> csn (DVE)
  -> tanh(c) (ACT) -> hs (DVE) -> next MM

Gate blocks are laid out [I@0, F@32, G@64, O@96] so ACT-A writes [ti;tf] and
ACT-B writes [g;to]; every elementwise product then has equal-base SBUF
operands (HW requirement) with NO extra alignment copies:
  u=(ti@0+1)*g@0, v=(tf@32+1)*cs@32, csn=u@0+0.5*v@0, hs=(to@32+1)*tc@32.
All gates tanh-unified (sigmoid(x)=(tanh(x/2)+1)/2); h,c stored doubled with
0.5 folded into host-built weights. Batch: 4096 -> 512/core -> 2 chains x 256
(2 groups of 128 in the free dim). Encoder merges its 2 layers into one
staggered lane; decoder runs cell0+cell1 serially (feedback forces it).
The x-projection is hoisted off the critical path: per 16-step window X is
transposed once per group (PE), and a 4-phase zero-padded weight trick lets a
single 32-row matmul prefetch Wih*x_t into the step's PSUM bank early.
fc bias rides in a 5th matmul row against a constant-1 row of the rh tile;
relu runs on gpsimd. State tiles are persistent and updated in place.
"""

import numpy as np
from contextlib import ExitStack

import concourse.bass as bass
import concourse.bacc as bacc
import concourse.tile as tile
import concourse.mybir as mybir
from concourse.bass_utils import run_bass_kernel_spmd

DT = mybir.dt.float32
AF = mybir.ActivationFunctionType
AO = mybir.AluOpType

B, T, F, H = 4096, 1024, 8, 4
NCORES = 8
BC = B // NCORES          # batch per core (512)
PB = 128                  # batch per group
S = 2                     # chains per core
NGc = 2                   # groups per chain
W = NGc * PB              # free width per chain op (256)
TW = 16                   # timesteps per DMA/transpose window
NEG = -60.0               # bias that forces tanh -> -1 (gate off)


def _blk(pg):
    # gate block base partition for pytorch gate pg (i,f,g,o -> 0,1,2,3)
    return 32 * pg


def build_consts(inp):
    f32 = np.float32
    eWih0, eWhh0, eb0 = inp["enc_Wih0"], inp["enc_Whh0"], inp["enc_b0"]
    eWih1, eWhh1, eb1 = inp["enc_Wih1"], inp["enc_Whh1"], inp["enc_b1"]
    dWih0, dWhh0, db0 = inp["dec_Wih0"], inp["dec_Whh0"], inp["dec_b0"]
    dWih1, dWhh1, db1 = inp["dec_Wih1"], inp["dec_Whh1"], inp["dec_b1"]
    fcW, fcb = inp["fc_W"], inp["fc_b"]

    ewhc = np.zeros((32, 128), f32)
    dwh0 = np.zeros((32, 128), f32)
    wcx = np.zeros((4, 128), f32)
    dwh1 = np.zeros((32, 128), f32)
    dwh1i = np.zeros((32, 128), f32)
    wcomp = dWih0 @ (0.5 * fcW)
    bshift = dWih0 @ fcb
    for pg in range(4):
        for u in range(H):
            gr = 4 * pg + u
            for L in range(2):
                m = _blk(pg) + L * 4 + u
                for k in range(H):
                    if L == 0:
                        ewhc[k, m] = 0.5 * eWhh0[gr, k]
                    else:
                        ewhc[k, m] = 0.5 * eWih1[gr, k]
                        ewhc[4 + k, m] = 0.5 * eWhh1[gr, k]
            md = _blk(pg) + u
            for k in range(H):
                dwh0[k, md] = 0.5 * dWhh0[gr, k]
                wcx[k, md] = wcomp[gr, k]
                dwh1[k, md] = 0.5 * dWhh1[gr, k]
                dwh1i[k, md] = 0.5 * dWih1[gr, k]

    # phase-p x-weights, replicated at both 32-row quads so the stationary
    # slice can share the moving operand's base partition (0 or 32 only;
    # matmul operands cannot start at partition 96)
    wxq = np.zeros((4, 64, 128), f32)
    for p in range(4):
        for q in range(2):
            for f in range(F):
                for pg in range(4):
                    for u in range(H):
                        wxq[p, 32 * q + 8 * p + f, _blk(pg) + u] = \
                            eWih0[4 * pg + u, f]

    # ACT bias tiles: A covers blocks I(rows 0:32),F(32:64); B covers G,O.
    def bias_tiles(b, layers, first_kill_l1=False, shift=None):
        bA = np.zeros((64, 1), f32)
        bB = np.zeros((64, 1), f32)
        for pg, arr, sc in ((0, bA, 0.5), (1, bA, 0.5), (2, bB, 1.0),
                            (3, bB, 0.5)):
            off = 0 if pg in (0, 2) else 32
            for L in range(layers):
                for u in range(H):
                    val = b[4 * pg + u]
                    if shift is not None:
                        val = val + shift[4 * pg + u]
                    arr[off + L * 4 + u, 0] = sc * val
                    if first_kill_l1 and L == 1 and pg in (0, 1):
                        arr[off + L * 4 + u, 0] = NEG
        return bA, bB

    ebias = eb0 * 0  # encoder per-layer biases
    bA = np.zeros((64, 1), f32)
    bB = np.zeros((64, 1), f32)
    bA0 = np.zeros((64, 1), f32)
    for pg, sc in ((0, 0.5), (1, 0.5), (2, 1.0), (3, 0.5)):
        arr = bA if pg in (0, 1) else bB
        arr0 = bA0 if pg in (0, 1) else bB
        off = 0 if pg in (0, 2) else 32
        for L in range(2):
            bsrc = eb0 if L == 0 else eb1
            for u in range(H):
                v = sc * bsrc[4 * pg + u]
                arr[off + L * 4 + u, 0] = v
        # step-0 variant: kill L1 input/forget gates so layer-1 state stays 0
        for L in range(2):
            bsrc = eb0 if L == 0 else eb1
            for u in range(H):
                v = sc * bsrc[4 * pg + u]
                if pg in (0, 1) and L == 1:
                    v = NEG
                arr0[off + L * 4 + u, 0] = v

    dbA0, dbB0 = bias_tiles(db0, 1, shift=bshift)
    dbA0f, dbB0f = bias_tiles(db0, 1)
    dbA1, dbB1 = bias_tiles(db1, 1)

    sB = np.zeros((64, 1), f32)
    sB[0:32, 0] = 1.0
    sB[32:64, 0] = 0.5

    wfc5 = np.zeros((32, 8), f32)
    for f in range(F):
        wfc5[0, f] = fcb[f]
    wfc4 = np.zeros((4, 8), f32)
    for u in range(H):
        for f in range(F):
            wfc4[u, f] = 0.5 * fcW[f, u]

    return {
        "ewhc": ewhc, "bA": bA, "bA0": bA0, "bB": bB, "sB": sB,
        "wxq0": wxq[0], "wxq1": wxq[1], "wxq2": wxq[2], "wxq3": wxq[3],
        "dwh0": dwh0, "wcx": wcx, "dwh1": dwh1, "dwh1i": dwh1i,
        "dbA0": dbA0, "dbB0": dbB0, "dbA0f": dbA0f, "dbB0f": dbB0f,
        "dbA1": dbA1, "dbB1": dbB1, "wfc5": wfc5, "wfc4": wfc4,
        "ident": np.eye(PB, dtype=f32),
    }


def const_shapes():
    shp = {
        "ewhc": (32, 128), "bA": (64, 1), "bA0": (64, 1), "bB": (64, 1),
        "sB": (64, 1),
        "dwh0": (32, 128), "wcx": (4, 128), "dwh1": (32, 128),
        "dwh1i": (32, 128),
        "dbA0": (64, 1), "dbB0": (64, 1), "dbA0f": (64, 1), "dbB0f": (64, 1),
        "dbA1": (64, 1), "dbB1": (64, 1), "wfc5": (32, 8), "wfc4": (4, 8),
        "ident": (PB, PB),
    }
    for p in range(4):
        shp[f"wxq{p}"] = (64, 128)
    return shp


def build_nc(Tl=T):
    nc = bacc.Bacc("TRN2", target_bir_lowering=False, debug=False)
    Xd = nc.dram_tensor("x", [BC, Tl, F], DT, kind="ExternalInput")
    Yd = nc.dram_tensor("y", [BC, Tl, F], DT, kind="ExternalOutput")
    cshapes = const_shapes()
    cdram = {k: nc.dram_tensor(k, list(s), DT, kind="ExternalInput")
             for k, s in cshapes.items()}

    def gb0(c, g):
        return (c * NGc + g) * PB

    with tile.TileContext(nc) as tc, ExitStack() as ctx:
        p = lambda name, bufs, **kw: ctx.enter_context(
            tc.tile_pool(name=name, bufs=bufs, **kw))
        wsb = p("wsb", 1)
        xsp = p("xs", 4)
        xtp = p("xt", 4)
        psZ = p("psZ", 4, space="PSUM")
        psT = p("psT", 1, space="PSUM")
        psY = p("psY", 1, space="PSUM")
        psO = p("psO", 2, space="PSUM")
        syp = p("sy", 2)
        obp = p("ob", 4)
        tga_p = p("tga", 2)
        tgb_p = p("tgb", 2)
        up = p("u", 2)
        vp = p("v", 2)
        tcp = p("tc", 2)
        hsp = p("hs", 3)
        csp = p("cs", 3)
        rhp = p("rh", 2)

        csb = {}
        for k, sshape in cshapes.items():
            t_ = wsb.tile(list(sshape), DT, name=f"c_{k}")
            nc.sync.dma_start(t_[:, :], cdram[k].ap()[:, :])
            csb[k] = t_
        ident = csb["ident"]
        wxq = [csb[f"wxq{q}"] for q in range(4)]

        one = wsb.tile([32, W], DT, name="one")
        nc.vector.memset(one[:, :], 0.0)
        nc.vector.memset(one[0:1, :], 1.0)

        def cell(c, pz, bAt, bBt, cs_old):
            """pz (PSUM) -> (new hs, new cs) via pooled tiles."""
            tgA = tga_p.tile([64, W], DT, name=f"tgA{c}")
            tgB = tgb_p.tile([64, W], DT, name=f"tgB{c}")
            nc.scalar.activation(tgA[:, :], pz[0:64, :], AF.Tanh,
                                 bias=bAt[:, 0:1], scale=0.5)
            nc.scalar.activation(tgB[:, :], pz[64:128, :], AF.Tanh,
                                 bias=bBt[:, 0:1], scale=csb["sB"][:, 0:1])
            V = vp.tile([32, W], DT, name=f"V{c}")
            nc.vector.scalar_tensor_tensor(
                V[:, :], tgA[32:64, :], 1.0, cs_old[32:64, :],
                AO.add, AO.mult)
            U = up.tile([32, W], DT, name=f"U{c}")
            nc.vector.scalar_tensor_tensor(
                U[:, :], tgA[0:32, :], 1.0, tgB[0:32, :],
                AO.add, AO.mult)
            cs_new = csp.tile([64, W], DT, name=f"cs{c}")
            nc.vector.scalar_tensor_tensor(
                cs_new[32:64, :], V[:, :], 0.5, U[:, :],
                AO.mult, AO.add)
            TC = tcp.tile([64, W], DT, name=f"TC{c}")
            nc.scalar.activation(TC[32:64, :], cs_new[32:64, :], AF.Tanh,
                                 bias=0.0, scale=0.5)
            hs_new = hsp.tile([32, W], DT, name=f"hs{c}")
            nc.vector.scalar_tensor_tensor(
                hs_new[0:32, :], tgB[32:64, :], 1.0, TC[32:64, :],
                AO.add, AO.mult)
            return hs_new, cs_new

        # ---------------- encoder ----------------
        HS, CS = [], []
        for c in range(S):
            hs0 = hsp.tile([32, W], DT, name=f"hs{c}")
            nc.vector.memset(hs0[:, :], 0.0)
            cs0 = csp.tile([64, W], DT, name=f"cs{c}")
            nc.vector.memset(cs0[:, :], 0.0)
            HS.append(hs0)
            CS.append(cs0)
        HD0 = [None] * S
        HD1 = [None] * S
        CD0 = [None] * S
        CD1 = [None] * S

        xt_cur = [None] * S
        for n in range(Tl + 1):
            if n < Tl and n % TW == 0:
                for c in range(S):
                    xtA = xtp.tile([64, W], DT, name=f"xtA{c}")
                    xtB = xtp.tile([64, W], DT, name=f"xtB{c}")
                    for g in range(NGc):
                        xs = xsp.tile([PB, TW * F], DT, name=f"xs{c}")
                        nc.sync.dma_start(
                            xs[:, :].rearrange("p (t f) -> p t f", f=F),
                            Xd.ap()[gb0(c, g):gb0(c, g) + PB, n:n + TW, :])
                        pT = psT.tile([128, 128], DT, name="pT")
                        nc.tensor.matmul(pT[:, :], xs[:, :], ident[:, :],
                                         is_transpose=True)
                        nc.scalar.copy(xtA[:, g * PB:(g + 1) * PB],
                                       pT[0:64, :])
                        nc.scalar.copy(xtB[:, g * PB:(g + 1) * PB],
                                       pT[64:128, :])
                    xt_cur[c] = (xtA, xtB)
            for c in range(S):
                pz = psZ.tile([128, W], DT, name="pz")
                if n < Tl:
                    tw, ph = n % TW, n % 4
                    xt = xt_cur[c][tw // 8]
                    q = (tw % 8) // 4
                    nc.tensor.matmul(pz[:, :],
                                     wxq[ph][32 * q:32 * q + 32, :],
                                     xt[32 * q:32 * q + 32, :],
                                     start=True, stop=False)
                nc.tensor.matmul(pz[:, :], csb["ewhc"][:, :], HS[c][:, :],
                                 start=(n == Tl), stop=True)
                bAt = csb["bA0"] if n == 0 else csb["bA"]
                HS[c], CS[c] = cell(c, pz, bAt, csb["bB"], CS[c])
                if n == Tl - 1:
                    HD0[c] = hsp.tile([32, W], DT, name=f"hs{c}")
                    nc.vector.memset(HD0[c][:, :], 0.0)
                    nc.sync.dma_start(HD0[c][0:4, :], HS[c][0:4, :])
                    CD0[c] = csp.tile([64, W], DT, name=f"cs{c}")
                    nc.vector.memset(CD0[c][:, :], 0.0)
                    nc.sync.dma_start(CD0[c][32:36, :], CS[c][32:36, :])
                if n == Tl:
                    HD1[c] = hsp.tile([32, W], DT, name=f"hs{c}")
                    nc.vector.memset(HD1[c][:, :], 0.0)
                    nc.sync.dma_start(HD1[c][0:4, :], HS[c][4:8, :])
                    CD1[c] = csp.tile([64, W], DT, name=f"cs{c}")
                    nc.vector.memset(CD1[c][:, :], 0.0)
                    nc.sync.dma_start(CD1[c][32:36, :], CS[c][36:40, :])

        # ---------------- decoder ----------------
        RH = [None] * S
        psO_cur = [None] * S
        for t in range(Tl):
            if t % TW == 0:
                for c in range(S):
                    psO_cur[c] = psO.tile([128, TW * 2 * F], DT, name="psO")
            jblk = TW - 1 - (t % TW)
            for c in range(S):
                pz0 = psZ.tile([128, W], DT, name="pz")
                nc.tensor.matmul(pz0[:, :], csb["dwh0"][:, :], HD0[c][:, :],
                                 start=True, stop=(t == 0))
                if t > 0:
                    nc.tensor.matmul(pz0[:, :], csb["wcx"][:, :],
                                     RH[c][0:4, :], start=False, stop=True)
                bA0t = csb["dbA0f"] if t == 0 else csb["dbA0"]
                bB0t = csb["dbB0f"] if t == 0 else csb["dbB0"]
                HD0[c], CD0[c] = cell(c, pz0, bA0t, bB0t, CD0[c])

                pz1 = psZ.tile([128, W], DT, name="pz")
                nc.tensor.matmul(pz1[:, :], csb["dwh1"][:, :], HD1[c][:, :],
                                 start=True, stop=False)
                nc.tensor.matmul(pz1[:, :], csb["dwh1i"][:, :], HD0[c][:, :],
                                 start=False, stop=True)
                HD1[c], CD1[c] = cell(c, pz1, csb["dbA1"], csb["dbB1"],
                                      CD1[c])

                rh = rhp.tile([32, W], DT, name=f"rh{c}")
                nc.gpsimd.tensor_scalar_max(rh[0:4, :], HD1[c][0:4, :], 0.0)
                RH[c] = rh
                py = psY.tile([8, W], DT, name="py")
                nc.tensor.matmul(py[:, :], csb["wfc5"][:, :], one[:, :],
                                 start=True, stop=False)
                nc.tensor.matmul(py[:, :], csb["wfc4"][:, :], rh[0:4, :],
                                 start=False, stop=True)
                sy = syp.tile([8, W], DT, name="sy")
                nc.scalar.copy(sy[:, :], py[:, :])
                for g in range(NGc):
                    nc.tensor.matmul(
                        psO_cur[c][:, jblk * 2 * F + g * F:
                                   jblk * 2 * F + (g + 1) * F],
                        sy[:, g * PB:(g + 1) * PB], ident[0:F, 0:F],
                        is_transpose=True)
            if t % TW == TW - 1:
                base = Tl - TW * (t // TW + 1)
                for c in range(S):
                    src = psO_cur[c][:, :].rearrange(
                        "p (t g f) -> p t g f", g=NGc, f=F)
                    for g in range(NGc):
                        ob = obp.tile([PB, TW * F], DT, name="ob")
                        nc.vector.tensor_copy(
                            ob[:, :].rearrange("p (t f) -> p t f", f=F),
                            src[:, :, g, :])
                        nc.sync.dma_start(
                            Yd.ap()[gb0(c, g):gb0(c, g) + PB,
                                    base:base + TW, :],
                            ob[:, :].rearrange("p (t f) -> p t f", f=F))
    nc.compile()
    return nc


_NC_CACHE = {}


def get_nc(Tl=T):
    if Tl not in _NC_CACHE:
        _NC_CACHE[Tl] = build_nc(Tl)
    return _NC_CACHE[Tl]


def kernel(**inputs):
    X = np.ascontiguousarray(np.asarray(inputs["X"], dtype=np.float32))
    Tl = X.shape[1]
    consts = build_consts({k: np.asarray(v, dtype=np.float32)
                           for k, v in inputs.items() if k != "X"})
    nc = get_nc(Tl)
    in_maps = []
    for core in range(NCORES):
        m = {"x": X[core * BC:(core + 1) * BC]}
        m.update(consts)
        in_maps.append(m)
    res = run_bass_kernel_spmd(nc, in_maps, core_ids=list(range(NCORES)))
    out = np.concatenate([r["y"] for r in res.results], axis=0)
    return out.astype(np.float32)
